# revision 2
# baseline (speedup 1.0000x reference)
"""Binary conv (XNOR-style) 3x3 + sync-BN on 8 Trainium2 NeuronCores.

Problem: x[32,256,56,56], w[256,256,3,3] -> sign(x) conv sign(w), pad 1,
then BatchNorm (training mode, global batch stats) with gamma/beta.

Sharding: data-parallel over batch (4 images per core, 8 cores). BN batch
stats are made global with a tiny (2 KB) AllGather of per-channel
sum / sum-of-squares (sync-BN), so the result matches single-device math.

Per-core kernel (v2 — restructured from the 167us baseline):
  - conv as shifted-window implicit GEMM on a zero-padded [58*58] fp8
    buffer; DoubleRow fp8 matmuls contract all 256 input channels at once.
    Moving operand is a 4D windowed AP [ci, 2, 8 rows stride 58, 56 cols]
    so each matmul computes 448 *valid* columns (no padded-column waste).
  - taps outer / 7 chunks inner per (img, cot) group; PSUM 7 banks for
    conv + 1 for warmup; one LDWEIGHTS per tap hides behind matmuls.
  - startup: image-0 is DMA'd in row-quarters and binarized on DVE (ct0)
    and GpSimd (ct1) in parallel as (x>=0)-0.5 = +-0.5 (rescaled 2x at
    PSUM copy); weights DMA'd + signed (ACT) in 3 tap-pieces; z pad
    borders memset once per buffer (interior overwritten per image);
    12 junk matmuls warm the PE HAM clock-gate before real work.
  - images 1-3 run cot-major (img1..3 cot0, then img1..3 cot1) so the
    cot0 BN stats AllGather + finalize + normalize + 6.4MB output DMA all
    overlap the cot1 conv stream; only cot1's output write is a tail.
  - per-channel sum via DVE evac accum; sum(y^2)/64 via ACT Square with
    fp32 accumulator; AllGather over 8 cores; rsqrt via
    reciprocal+sqrt+Newton; y*scale+bias on DVE/ACT/GpSimd in half-image
    tiles -> fp32 out.
"""

import os
import numpy as np

import concourse.bass as bass
import concourse.mybir as mybir
import concourse.tile as tile
from concourse import bacc
from concourse import bass_utils

F32 = mybir.dt.float32
F16 = mybir.dt.float16
F8 = mybir.dt.float8e4

N_CORES = 8
NL = 4            # images per core
CI = 256          # input channels
CO = 256          # output channels
H = W = 56
HP = 58           # padded row length
CR = 8            # image rows per chunk
NCHUNK = 7        # 7 chunks x 8 rows = 56 rows
VCHUNK = CR * W   # 448 matmul columns per chunk (all valid)
VLEN = NCHUNK * VCHUNK  # 3136
ZROWS = 58        # padded rows (top pad + 56 + bottom pad)
ZPAD = 3376       # fp8 per-ci-tile stride; >= 58*58+2 read slack, %16==0
HHALF = 28        # output norm/store granularity (half image rows)
HLEN = HHALF * W  # 1568
NTOT_PIX = 32 * H * W    # BN normalizer (full batch)
BN_EPS = 1e-5
SSQ_SCALE = 1.0 / 64.0  # keep y^2/64 in fp16 range in the junk output
DR = mybir.MatmulPerfMode.DoubleRow


def _build(timing_proxy: bool = False):
    nc = bacc.Bacc("TRN2", target_bir_lowering=False, debug=False,
                   num_devices=N_CORES)

    xs = nc.dram_tensor("xs", [NL, CI, H, W], F32, kind="ExternalInput").ap()
    wt = nc.dram_tensor("wt", [CI, 9, CO], F32, kind="ExternalInput").ap()
    gamma = nc.dram_tensor("gamma", [CO], F32, kind="ExternalInput").ap()
    beta = nc.dram_tensor("beta", [CO], F32, kind="ExternalInput").ap()
    o = nc.dram_tensor("o", [NL, CO, H, W], F32, kind="ExternalOutput").ap()

    with tile.TileContext(nc) as tc:
        with (
            tc.tile_pool(name="wpool", bufs=1) as wpool,
            tc.tile_pool(name="xpool", bufs=1) as xpool,
            tc.tile_pool(name="zpool", bufs=1) as zpool,
            tc.tile_pool(name="ypool", bufs=1) as ypool,
            tc.tile_pool(name="spool", bufs=1) as spool,
            tc.tile_pool(name="jpool", bufs=1) as jpool,
            tc.tile_pool(name="opool", bufs=1) as opool,
            tc.tile_pool(name="psum", bufs=1, space="PSUM") as psum_pool,
            tc.tile_pool(name="dram", bufs=1, space="DRAM") as dram,
        ):
            # ---- PE warmup: junk matmuls on a zeroed tile keep the HAM
            # clock-gate busy from ~7us so the real stream starts at 2.4GHz
            jk = spool.tile([128, 2, 256], F8, tag="jk")
            nc.vector.memset(jk[:], 0.0)
            jkacc = psum_pool.tile([128, 256], F32, tag="jkps")
            for i in range(12):
                nc.tensor.matmul(jkacc[:], jk[:, :, 0:128], jk[:], start=True,
                                 stop=True, perf_mode=DR)

            # preload the sqrt ACT table set off the critical path
            sqwarm = spool.tile([128, 1], F32, tag="sqwarm")
            nc.vector.memset(sqwarm[:], 1.0)
            nc.scalar.sqrt(sqwarm[:], sqwarm[:])

            gath = spool.tile([128, 2, N_CORES, 2], F32, tag="gath")
            nc.vector.memset(gath[:], 0.0)  # proxy mode only fills rank 0

            # ---- weights: DMA + ACT sign in 3-tap pieces, interleaved with
            # image-0 quarters so tap t is signed before the sweep needs it
            w_f32 = wpool.tile([128, 2, 9, CO], F32, tag="wf32")
            w_bin = wpool.tile([128, 2, 9, CO], F8, tag="wbin")
            wt_r = wt.rearrange("(ct p) t co -> p ct t co", p=128)

            # ---- z: 3 persistent padded fp8 buffers [p, buf, ci_tile, flat]
            # imgs 0,3 share slot 0; img1 slot 1; img2 slot 2. Pad borders
            # (row 0, row 57+slack, col triples) memset once; interior is
            # fully rewritten per image.
            zz = zpool.tile([128, 3, 2, ZPAD], F8, tag="zz")

            def pad_memset(b):
                nc.gpsimd.memset(zz[:, b, :, 0:HP], 0.0)
                trip = zz[:, b, :, 57:57 + 57 * HP].rearrange(
                    "p k (r t) -> p k r t", t=HP)[:, :, :, 0:3]
                nc.gpsimd.memset(trip, 0.0)
                nc.gpsimd.memset(zz[:, b, :, 57 * HP:ZPAD], 0.0)

            nc.sync.dma_start(w_f32[:, :, 0:3, :], wt_r[:, :, 0:3, :])

            # image 0: row-quarter DMAs, binarize ct0 on DVE / ct1 on GpSimd
            # as (x>=0)-0.5 = +-0.5 (fp8-exact); PSUM copy rescales by 2.
            pad_memset(0)
            QR = 14
            z58_0 = {ct: zz[:, 0, ct, 0:ZROWS * HP].rearrange(
                "p (r q) -> p r q", q=HP) for ct in range(2)}
            for q in range(4):
                for ct in range(2):
                    xq = xpool.tile([128, QR, W], F32, tag="xq", bufs=8,
                                    name=f"xq_{q}_{ct}")
                    nc.sync.dma_start(
                        xq[:], xs[0, ct * 128:(ct + 1) * 128,
                                  QR * q:QR * (q + 1)])
                    dst = z58_0[ct][:, 1 + QR * q:1 + QR * (q + 1), 2:58]
                    eng = nc.vector if ct == 0 else nc.gpsimd
                    eng.tensor_scalar(dst, xq[:], 0.0, 0.5,
                                      op0=mybir.AluOpType.is_ge,
                                      op1=mybir.AluOpType.subtract)
                if q == 0:
                    nc.sync.dma_start(w_f32[:, :, 3:6, :], wt_r[:, :, 3:6, :])
                if q == 1:
                    nc.sync.dma_start(w_f32[:, :, 6:9, :], wt_r[:, :, 6:9, :])
            pad_memset(1)
            pad_memset(2)

            nc.scalar.sign(w_bin[:, :, 0:1, :], w_f32[:, :, 0:1, :])
            nc.scalar.sign(w_bin[:, :, 1:3, :], w_f32[:, :, 1:3, :])
            nc.scalar.sign(w_bin[:, :, 3:6, :], w_f32[:, :, 3:6, :])
            nc.scalar.sign(w_bin[:, :, 6:9, :], w_f32[:, :, 6:9, :])

            # gamma/beta per-partition: channel c = t*128 + p
            gb_g = spool.tile([128, 2], F32, tag="gb_g")
            gb_b = spool.tile([128, 2], F32, tag="gb_b")

            # ---- persistent state ----
            ys = ypool.tile([128, 2, NL, VLEN], F16, tag="ys")
            sums = spool.tile([128, 2, NL, NCHUNK], F32, tag="sums")
            ssqa = spool.tile([128, 2, NL - 1 + NCHUNK], F32, tag="ssqa")

            jk2 = spool.tile([128, NL * NCHUNK], F32, tag="jk2")
            sums_b = spool.tile([128, 2, NL, NCHUNK], F32, tag="sums_b")
            scbs = {}

            ZSLOT = {0: 0, 1: 1, 2: 2, 3: 0}

            def zwin(n, c, kh, kw):
                """Moving operand for chunk c, tap (kh,kw): [p, 2, 8, 56]
                rows stride 58 in the padded buffer."""
                base = HP * (CR * c + kh) + 1 + kw
                zs = zz[:, ZSLOT[n], :, base:base + 464]
                return zs.rearrange("p k (r q) -> p k r q", q=HP)[:, :, :, 0:W]

            def conv_group(n, cot):
                """9-tap sweep over all 7 chunks of (img n, cot); evac to ys
                with per-chunk channel sums; img3 also squares per chunk."""
                cos = slice(cot * 128, (cot + 1) * 128)
                accs = {}
                for c in range(NCHUNK):
                    accs[c] = psum_pool.tile([128, VCHUNK], F32, tag="acc",
                                             bufs=7, name=f"acc_{n}_{cot}_{c}")
                for t in range(9):
                    kh, kw = t // 3, t % 3
                    for c in range(NCHUNK):
                        nc.tensor.matmul(
                            accs[c][:],
                            w_bin[:, :, t, cos],
                            zwin(n, c, kh, kw),
                            start=(t == 0), stop=(t == 8),
                            perf_mode=DR,
                        )
                for c in range(NCHUNK):
                    dst = ys[:, cot, n, VCHUNK * c:VCHUNK * (c + 1)]
                    nc.vector.tensor_scalar(
                        dst, accs[c][:], 2.0 if n == 0 else 1.0, 0.0,
                        op0=mybir.AluOpType.mult,
                        op1=mybir.AluOpType.add,
                        accum_out=sums[:, cot, n, c:c + 1])
                    if n == NL - 1:
                        # last image: chunk-granular squares so the
                        # post-conv stats tail is tiny (per cot)
                        junk = jpool.tile([128, VCHUNK], F16, tag="junkc",
                                          bufs=2, name=f"junkc_{cot}_{c}")
                        nc.scalar.activation(
                            junk[:], dst,
                            mybir.ActivationFunctionType.Square,
                            scale=0.125,
                            accum_out=ssqa[:, cot, NL - 1 + c:NL + c])

            def emit_ssq(n, cot):
                # per-image sum(y^2): Square(y/8) w/ fp32 accum = sum(y^2)/64
                junk = jpool.tile([128, VLEN], F16, tag="junkb", bufs=2,
                                  name=f"junkb_{n}_{cot}")
                nc.scalar.activation(
                    junk[:], ys[:, cot, n, :],
                    mybir.ActivationFunctionType.Square,
                    scale=0.125,
                    accum_out=ssqa[:, cot, n:n + 1])

            def emit_stats(cot):
                # fold local stats: [sum, sum(y^2)/64] over imgs/chunks.
                # Cross-engine reads of accum_out tiles fault this HW, so
                # each accum tile gets an engine-local barrier copy first.
                cc_stage = spool.tile([128, 2], F32, tag=f"cc_stage{cot}",
                                      name=f"cc_stage_{cot}")
                nc.vector.tensor_copy(sums_b[:, cot], sums[:, cot])
                nc.scalar.activation(
                    jk2[:], sums_b[:, cot, :, :],
                    mybir.ActivationFunctionType.Copy,
                    accum_out=cc_stage[:, 0:1])
                nc.scalar.activation(
                    jk2[:, 0:NL - 1 + NCHUNK], ssqa[:, cot, :],
                    mybir.ActivationFunctionType.Copy,
                    scale=1.0 / SSQ_SCALE,
                    accum_out=cc_stage[:, 1:2])
                cc_stage2 = spool.tile([128, 2], F32, tag=f"cc_stage2{cot}",
                                       name=f"cc_stage2_{cot}")
                nc.scalar.copy(cc_stage2[:], cc_stage[:])
                cc_in = dram.tile([128, 2], F32, tag=f"cc_in{cot}",
                                  name=f"cc_in_{cot}")
                cc_out = dram.tile([N_CORES * 128, 2], F32,
                                   tag=f"cc_out{cot}", name=f"cc_out_{cot}")
                nc.sync.dma_start(cc_in[:], cc_stage2[:])
                if timing_proxy:
                    nc.sync.dma_start(cc_out[0:128, :], cc_in[:])
                    nc.sync.dma_start(gath[:, cot, 0, :], cc_out[0:128, :])
                else:
                    nc.gpsimd.collective_compute(
                        "AllGather",
                        mybir.AluOpType.bypass,
                        replica_groups=[list(range(N_CORES))],
                        ins=[cc_in.opt()],
                        outs=[cc_out.opt()],
                    )
                    nc.sync.dma_start(
                        gath[:, cot],
                        cc_out.rearrange("(r p) s -> p r s", p=128))

            def emit_finalize(cot):
                gstat = spool.tile([128, 2], F32, tag=f"gstat{cot}",
                                   name=f"gstat_{cot}")
                # [128, 8, 2] -> sum over ranks per stat
                nc.vector.reduce_sum(
                    gstat[:], gath[:, cot].rearrange("p r s -> p s r"),
                    axis=mybir.AxisListType.X)
                mv = spool.tile([128, 2], F32, tag=f"mv{cot}",
                                name=f"mv_{cot}")
                mean, ey2e = mv[:, 0:1], mv[:, 1:2]
                var = spool.tile([128, 1], F32, tag=f"var{cot}",
                                 name=f"var_{cot}")
                r0 = spool.tile([128, 1], F32, tag=f"r0{cot}",
                                name=f"r0_{cot}")
                t1 = spool.tile([128, 1], F32, tag=f"t1{cot}",
                                name=f"t1_{cot}")
                sc = spool.tile([128, 1], F32, tag=f"sc{cot}",
                                name=f"sc_{cot}")
                bs = spool.tile([128, 1], F32, tag=f"bs{cot}",
                                name=f"bs_{cot}")
                nc.vector.tensor_scalar_mul(mean, gstat[:, 0:1],
                                            1.0 / NTOT_PIX)
                nc.vector.tensor_scalar(ey2e, gstat[:, 1:2],
                                        1.0 / NTOT_PIX, BN_EPS,
                                        op0=mybir.AluOpType.mult,
                                        op1=mybir.AluOpType.add)
                nc.vector.tensor_tensor(var[:], mean, mean,
                                        op=mybir.AluOpType.mult)
                nc.vector.tensor_tensor(var[:], ey2e, var[:],
                                        op=mybir.AluOpType.subtract)
                # inv = rsqrt(var+eps): sqrt(1/v) then one Newton step
                nc.vector.reciprocal(r0[:], var[:])
                nc.scalar.sqrt(r0[:], r0[:])
                nc.vector.tensor_tensor(t1[:], r0[:], r0[:],
                                        op=mybir.AluOpType.mult)
                nc.vector.tensor_tensor(t1[:], t1[:], var[:],
                                        op=mybir.AluOpType.mult)
                nc.vector.tensor_scalar(t1[:], t1[:], -0.5, 1.5,
                                        op0=mybir.AluOpType.mult,
                                        op1=mybir.AluOpType.add)
                nc.vector.tensor_tensor(r0[:], r0[:], t1[:],
                                        op=mybir.AluOpType.mult)
                nc.vector.tensor_tensor(sc[:], gb_g[:, cot:cot + 1], r0[:],
                                        op=mybir.AluOpType.mult)
                nc.vector.tensor_tensor(t1[:], mean, sc[:],
                                        op=mybir.AluOpType.mult)
                nc.vector.tensor_tensor(bs[:], gb_b[:, cot:cot + 1], t1[:],
                                        op=mybir.AluOpType.subtract)
                scbs[cot] = (sc, bs)

            def emit_norm(cot, engines):
                """normalize + store in half-image tiles; engines is an
                8-long list over (img, half)."""
                sc, bs = scbs[cot]
                for i, (n, hh) in enumerate(
                        (n, hh) for n in range(NL) for hh in range(2)):
                    ost = opool.tile([128, HHALF, W], F32, tag="ost", bufs=4,
                                     name=f"ost_{cot}_{n}_{hh}")
                    yv = ys[:, cot, n, HLEN * hh:HLEN * (hh + 1)].rearrange(
                        "p (r q) -> p r q", q=W)
                    eng = engines[i]
                    if eng == "act":
                        nc.scalar.activation(
                            ost[:], yv,
                            mybir.ActivationFunctionType.Identity,
                            bias=bs[:], scale=sc[:])
                    else:
                        e = nc.vector if eng == "dve" else nc.gpsimd
                        e.tensor_scalar(
                            ost[:], yv, sc[:], bs[:],
                            op0=mybir.AluOpType.mult,
                            op1=mybir.AluOpType.add)
                    nc.sync.dma_start(
                        o[n, cot * 128:(cot + 1) * 128,
                          HHALF * hh:HHALF * (hh + 1), :],
                        ost[:])

            def build_z(n):
                """images 1-3: whole-image DMA then ACT sign into z."""
                for ct in range(2):
                    xst = xpool.tile([128, H, W], F32, tag="xst", bufs=2,
                                     name=f"xst_{n}_{ct}")
                    nc.sync.dma_start(xst[:], xs[n, ct * 128:(ct + 1) * 128])
                    z58 = zz[:, ZSLOT[n], ct, 0:ZROWS * HP].rearrange(
                        "p (r q) -> p r q", q=HP)
                    nc.scalar.sign(z58[:, 1:57, 2:58], xst[:])

            # ================= schedule =================
            # image 0: image-major (cot0 then cot1) while the rest streams in
            conv_group(0, 0)
            emit_ssq(0, 0)
            build_z(1)
            nc.sync.dma_start(gb_g[:], gamma.rearrange("(t p) -> p t", p=128))
            nc.sync.dma_start(gb_b[:], beta.rearrange("(t p) -> p t", p=128))
            conv_group(0, 1)
            emit_ssq(0, 1)
            build_z(2)

            # images 1-3: cot-major. cot0 stats/norm/store overlap cot1 conv.
            conv_group(1, 0)
            emit_ssq(1, 0)
            build_z(3)
            conv_group(2, 0)
            emit_ssq(2, 0)
            conv_group(3, 0)

            emit_stats(0)
            emit_finalize(0)
            emit_norm(0, ["gpsimd", "act", "gpsimd", "dve",
                          "gpsimd", "act", "gpsimd", "dve"])

            conv_group(1, 1)
            emit_ssq(1, 1)
            conv_group(2, 1)
            emit_ssq(2, 1)
            conv_group(3, 1)

            emit_stats(1)
            emit_finalize(1)
            emit_norm(1, ["gpsimd", "dve", "act", "gpsimd",
                          "dve", "act", "gpsimd", "gpsimd"])

    nc.compile()
    return nc


_CACHE: dict = {}


def _get_nc():
    key = "proxy" if os.environ.get("BK_TIMING_PROXY") == "1" else "real"
    if key not in _CACHE:
        _CACHE[key] = _build(timing_proxy=(key == "proxy"))
    return _CACHE[key]


def kernel(x, w, gamma, beta):
    x = np.ascontiguousarray(np.asarray(x, dtype=np.float32))
    w = np.asarray(w, dtype=np.float32)
    gamma = np.ascontiguousarray(np.asarray(gamma, dtype=np.float32))
    beta = np.ascontiguousarray(np.asarray(beta, dtype=np.float32))
    # host-side layout only (no math): [co,ci,kh,kw] -> [ci, kh*kw, co]
    w_t = np.ascontiguousarray(w.transpose(1, 2, 3, 0).reshape(CI, 9, CO))

    nc = _get_nc()
    in_maps = [
        {"xs": x[NL * c:NL * (c + 1)], "wt": w_t, "gamma": gamma, "beta": beta}
        for c in range(N_CORES)
    ]
    res = bass_utils.run_bass_kernel_spmd(
        nc, in_maps, core_ids=list(range(N_CORES)))
    return np.concatenate([res.results[c]["o"] for c in range(N_CORES)], axis=0)


# revision 7
# speedup vs baseline: 1.1149x; 1.1149x over previous
"""Binary conv (XNOR-style) 3x3 + sync-BN on 8 Trainium2 NeuronCores.

Problem: x[32,256,56,56], w[256,256,3,3] -> sign(x) conv sign(w), pad 1,
then BatchNorm (training mode, global batch stats) with gamma/beta.

Sharding: data-parallel over batch (4 images per core, 8 cores). BN batch
stats are made global with a tiny (2 KB) AllGather of per-channel
sum / sum-of-squares (sync-BN), so the result matches single-device math.

Per-core kernel (v3):
  - conv as shifted-window implicit GEMM on a zero-padded [58*58] fp8
    buffer; DoubleRow fp8 matmuls contract all 256 input channels at once.
    Moving operand is a 4D windowed AP [ci, 2, 8 rows stride 58, 56 cols]
    so each matmul computes 448 *valid* columns (no padded-column waste).
  - taps outer / chunks inner in 4+3 half-groups; consecutive groups use
    disjoint PSUM bank quads (bufs=8) so bank WAR never stalls the PE.
    One LDWEIGHTS per tap hides behind matmuls (background weight buffer).
  - startup: ~24 junk matmuls on a zeroed tile warm the PE HAM clock-gate;
    image 0 is DMA'd in row-quarters and binarized on DVE as
    (x>=0)-0.5 = +-0.5 (fp8-exact; rescaled 2x at PSUM evac) fully before
    its conv starts -- writing a z buffer the PE is streaming runs 10-20x
    slow, so build and consume never overlap a buffer. Weights are DMA'd
    + signed (ACT) in 3-tap pieces interleaved with the quarters; z pad
    borders memset once per buffer (interiors are overwritten per image).
  - images 1-3 run cot-major (img1..3 cot0, then img1..3 cot1) so the
    cot0 BN stats AllGather + finalize + normalize + 6.4MB output DMA all
    overlap the cot1 conv stream; only cot1's output write is a tail.
    All signs are emitted early on ACT (engine FIFOs are NOT reordered).
  - per-channel sum via DVE evac accum; sum(y^2)/64 via ACT Square with
    fp32 accumulator (big pass per img-cot; final image per-chunk so the
    stats tail is short); AllGather over 8 cores; rsqrt via
    reciprocal+sqrt+Newton; y*scale+bias on DVE/ACT in half-image tiles
    -> fp32 out.
"""

import os
import numpy as np

import concourse.bass as bass
import concourse.mybir as mybir
import concourse.tile as tile
from concourse import bacc
from concourse import bass_utils

F32 = mybir.dt.float32
F16 = mybir.dt.float16
F8 = mybir.dt.float8e4

N_CORES = 8
NL = 4            # images per core
CI = 256          # input channels
CO = 256          # output channels
H = W = 56
HP = 58           # padded row length
CR = 8            # image rows per chunk
NCHUNK = 7        # 7 chunks x 8 rows = 56 rows
VCHUNK = CR * W   # 448 matmul columns per chunk (all valid)
VLEN = NCHUNK * VCHUNK  # 3136
ZROWS = 58        # padded rows (top pad + 56 + bottom pad)
ZPAD = 3376       # fp8 per-ci-tile stride; >= 58*58+2 read slack, %16==0
HHALF = 28        # output norm/store granularity (half image rows)
HLEN = HHALF * W  # 1568
NTOT_PIX = 32 * H * W    # BN normalizer (full batch)
BN_EPS = 1e-5
SSQ_SCALE = 1.0 / 64.0  # keep y^2/64 in fp16 range in the junk output
DR = mybir.MatmulPerfMode.DoubleRow


def _build(timing_proxy: bool = False):
    nc = bacc.Bacc("TRN2", target_bir_lowering=False, debug=False,
                   num_devices=N_CORES)

    xs = nc.dram_tensor("xs", [NL, CI, H, W], F32, kind="ExternalInput").ap()
    wt = nc.dram_tensor("wt", [CI, 9, CO], F32, kind="ExternalInput").ap()
    gamma = nc.dram_tensor("gamma", [CO], F32, kind="ExternalInput").ap()
    beta = nc.dram_tensor("beta", [CO], F32, kind="ExternalInput").ap()
    o = nc.dram_tensor("o", [NL, CO, H, W], F32, kind="ExternalOutput").ap()

    with tile.TileContext(nc) as tc:
        with (
            tc.tile_pool(name="wpool", bufs=1) as wpool,
            tc.tile_pool(name="xpool", bufs=1) as xpool,
            tc.tile_pool(name="zpool", bufs=1) as zpool,
            tc.tile_pool(name="ypool", bufs=1) as ypool,
            tc.tile_pool(name="spool", bufs=1) as spool,
            tc.tile_pool(name="jpool", bufs=1) as jpool,
            tc.tile_pool(name="opool", bufs=1) as opool,
            tc.tile_pool(name="psum", bufs=8, space="PSUM") as psum_pool,
            tc.tile_pool(name="dram", bufs=1, space="DRAM") as dram,
        ):
            # ---- PE warmup: junk matmuls on a zeroed tile keep the HAM
            # clock-gate busy from ~7us so the real stream starts at 2.4GHz.
            # They draw PSUM banks from the same rotation as the conv accs.
            jk = spool.tile([128, 2, 256], F8, tag="jk")
            nc.vector.memset(jk[:], 0.0)
            for i in range(24):
                jkacc = psum_pool.tile([128, 256], F32, tag="acc",
                                       name=f"jkacc_{i}")
                nc.tensor.matmul(jkacc[:], jk[:, :, 0:128], jk[:], start=True,
                                 stop=True, perf_mode=DR)

            # preload the sqrt ACT table set off the critical path
            sqwarm = spool.tile([128, 1], F32, tag="sqwarm")
            nc.vector.memset(sqwarm[:], 1.0)
            nc.scalar.sqrt(sqwarm[:], sqwarm[:])

            gath = spool.tile([128, 2, N_CORES, 2], F32, tag="gath")
            nc.vector.memset(gath[:], 0.0)  # proxy mode only fills rank 0

            # ---- weights: DMA + ACT sign in 3-tap pieces, interleaved with
            # image-0 quarters so tap t is signed before the sweep needs it
            w_f32 = wpool.tile([128, 2, 9, CO], F32, tag="wf32")
            w_bin = wpool.tile([128, 2, 9, CO], F8, tag="wbin")
            wt_r = wt.rearrange("(ct p) t co -> p ct t co", p=128)

            # ---- z: 3 persistent padded fp8 buffers [p, buf, ci_tile, flat]
            # imgs 0,3 share slot 0; img1 slot 1; img2 slot 2. Pad borders
            # (row 0, row 57+slack, col triples) memset once; interior is
            # fully rewritten per image.
            zz = zpool.tile([128, 3, 2, ZPAD], F8, tag="zz")

            def pad_memset(b):
                nc.gpsimd.memset(zz[:, b, :, 0:HP], 0.0)
                trip = zz[:, b, :, 57:57 + 57 * HP].rearrange(
                    "p k (r t) -> p k r t", t=HP)[:, :, :, 0:3]
                nc.gpsimd.memset(trip, 0.0)
                nc.gpsimd.memset(zz[:, b, :, 57 * HP:ZPAD], 0.0)

            nc.sync.dma_start(w_f32[:, :, 0:3, :], wt_r[:, :, 0:3, :])

            # image 0: row-quarter DMAs, binarize on DVE as (x>=0)-0.5.
            # z0 is built completely before image 0's conv begins.
            pad_memset(0)
            QR = 14
            z58_0 = {ct: zz[:, 0, ct, 0:ZROWS * HP].rearrange(
                "p (r q) -> p r q", q=HP) for ct in range(2)}
            for q in range(4):
                for ct in range(2):
                    xq = xpool.tile([128, QR, W], F32, tag="xq", bufs=8,
                                    name=f"xq_{q}_{ct}")
                    nc.sync.dma_start(
                        xq[:], xs[0, ct * 128:(ct + 1) * 128,
                                  QR * q:QR * (q + 1)])
                    dst = z58_0[ct][:, 1 + QR * q:1 + QR * (q + 1), 2:58]
                    nc.vector.tensor_scalar(dst, xq[:], 0.0, 0.5,
                                            op0=mybir.AluOpType.is_ge,
                                            op1=mybir.AluOpType.subtract)
                if q == 0:
                    nc.sync.dma_start(w_f32[:, :, 3:6, :], wt_r[:, :, 3:6, :])
                if q == 1:
                    nc.sync.dma_start(w_f32[:, :, 6:9, :], wt_r[:, :, 6:9, :])
            pad_memset(1)
            pad_memset(2)

            nc.scalar.sign(w_bin[:, :, 0:1, :], w_f32[:, :, 0:1, :])
            nc.scalar.sign(w_bin[:, :, 1:3, :], w_f32[:, :, 1:3, :])
            nc.scalar.sign(w_bin[:, :, 3:6, :], w_f32[:, :, 3:6, :])
            nc.scalar.sign(w_bin[:, :, 6:9, :], w_f32[:, :, 6:9, :])

            # gamma/beta per-partition: channel c = t*128 + p
            gb_g = spool.tile([128, 2], F32, tag="gb_g")
            gb_b = spool.tile([128, 2], F32, tag="gb_b")

            # ---- persistent state ----
            ys = ypool.tile([128, 2, NL, VLEN], F16, tag="ys")
            sums = spool.tile([128, 2, NL, NCHUNK], F32, tag="sums")
            ssqa = spool.tile([128, 2, NL - 1 + NCHUNK], F32, tag="ssqa")

            jk2 = spool.tile([128, NL * NCHUNK], F32, tag="jk2")
            sums_b = spool.tile([128, 2, NL, NCHUNK], F32, tag="sums_b")
            scbs = {}

            ZSLOT = {0: 0, 1: 1, 2: 2, 3: 0}

            def build_z(n):
                """images 1-3: whole-image DMA then ACT sign into z."""
                for ct in range(2):
                    xst = xpool.tile([128, H, W], F32, tag="xst", bufs=3,
                                     name=f"xst_{n}_{ct}")
                    nc.sync.dma_start(xst[:], xs[n, ct * 128:(ct + 1) * 128])
                    z58 = zz[:, ZSLOT[n], ct, 0:ZROWS * HP].rearrange(
                        "p (r q) -> p r q", q=HP)
                    nc.scalar.sign(z58[:, 1:57, 2:58], xst[:])

            build_z(1)
            nc.sync.dma_start(gb_g[:], gamma.rearrange("(t p) -> p t", p=128))
            nc.sync.dma_start(gb_b[:], beta.rearrange("(t p) -> p t", p=128))
            build_z(2)

            def zwin(n, c, kh, kw):
                """Moving operand for chunk c, tap (kh,kw): [p, 2, 8, 56]
                rows stride 58 in the padded buffer."""
                base = HP * (CR * c + kh) + 1 + kw
                zs = zz[:, ZSLOT[n], :, base:base + 464]
                return zs.rearrange("p k (r q) -> p k r q", q=HP)[:, :, :, 0:W]

            def conv_group(n, cot, rev=False):
                """two 9-tap half-group sweeps (chunks 0-3, 4-6); evac to ys
                with per-chunk channel sums; img3 also squares per chunk.
                rev=True gates the first matmul on the last-written z
                quarter (never stream a z buffer while it's being built)."""
                cos = slice(cot * 128, (cot + 1) * 128)
                halves = ((6, 5, 4, 3), (2, 1, 0)) if rev else \
                    (range(0, 4), range(4, NCHUNK))
                for chunks in halves:
                    accs = {
                        c: psum_pool.tile([128, VCHUNK], F32, tag="acc",
                                          name=f"acc_{n}_{cot}_{c}")
                        for c in chunks
                    }
                    for t in range(9):
                        kh, kw = t // 3, t % 3
                        for c in chunks:
                            nc.tensor.matmul(
                                accs[c][:],
                                w_bin[:, :, t, cos],
                                zwin(n, c, kh, kw),
                                start=(t == 0), stop=(t == 8),
                                perf_mode=DR,
                            )
                    for c in chunks:
                        dst = ys[:, cot, n, VCHUNK * c:VCHUNK * (c + 1)]
                        nc.vector.tensor_scalar(
                            dst, accs[c][:], 2.0 if n == 0 else 1.0, 0.0,
                            op0=mybir.AluOpType.mult,
                            op1=mybir.AluOpType.add,
                            accum_out=sums[:, cot, n, c:c + 1])
                        if n == NL - 1:
                            # last image: chunk-granular squares so the
                            # post-conv stats tail is tiny (per cot)
                            junk = jpool.tile(
                                [128, VCHUNK], F16, tag="junkc", bufs=2,
                                name=f"junkc_{cot}_{c}")
                            nc.scalar.activation(
                                junk[:], dst,
                                mybir.ActivationFunctionType.Square,
                                scale=0.125,
                                accum_out=ssqa[:, cot, NL - 1 + c:NL + c])

            def emit_ssq(n, cot):
                # per-image sum(y^2): Square(y/8) w/ fp32 accum = sum(y^2)/64
                junk = jpool.tile([128, VLEN], F16, tag="junkb", bufs=2,
                                  name=f"junkb_{n}_{cot}")
                nc.scalar.activation(
                    junk[:], ys[:, cot, n, :],
                    mybir.ActivationFunctionType.Square,
                    scale=0.125,
                    accum_out=ssqa[:, cot, n:n + 1])

            def emit_stats(cot):
                # fold local stats: [sum, sum(y^2)/64] over imgs/chunks.
                # Cross-engine reads of accum_out tiles fault this HW, so
                # each accum tile gets an engine-local barrier copy first.
                cc_stage = spool.tile([128, 2], F32, tag=f"cc_stage{cot}",
                                      name=f"cc_stage_{cot}")
                nc.vector.tensor_copy(sums_b[:, cot], sums[:, cot])
                nc.scalar.activation(
                    jk2[:], sums_b[:, cot, :, :],
                    mybir.ActivationFunctionType.Copy,
                    accum_out=cc_stage[:, 0:1])
                nc.scalar.activation(
                    jk2[:, 0:NL - 1 + NCHUNK], ssqa[:, cot, :],
                    mybir.ActivationFunctionType.Copy,
                    scale=1.0 / SSQ_SCALE,
                    accum_out=cc_stage[:, 1:2])
                cc_stage2 = spool.tile([128, 2], F32, tag=f"cc_stage2{cot}",
                                       name=f"cc_stage2_{cot}")
                nc.scalar.copy(cc_stage2[:], cc_stage[:])
                cc_in = dram.tile([128, 2], F32, tag=f"cc_in{cot}",
                                  name=f"cc_in_{cot}")
                cc_out = dram.tile([N_CORES * 128, 2], F32,
                                   tag=f"cc_out{cot}", name=f"cc_out_{cot}")
                nc.sync.dma_start(cc_in[:], cc_stage2[:])
                if timing_proxy:
                    nc.sync.dma_start(cc_out[0:128, :], cc_in[:])
                    nc.sync.dma_start(gath[:, cot, 0, :], cc_out[0:128, :])
                else:
                    nc.gpsimd.collective_compute(
                        "AllGather",
                        mybir.AluOpType.bypass,
                        replica_groups=[list(range(N_CORES))],
                        ins=[cc_in.opt()],
                        outs=[cc_out.opt()],
                    )
                    nc.sync.dma_start(
                        gath[:, cot],
                        cc_out.rearrange("(r p) s -> p r s", p=128))

            def emit_finalize(cot):
                gstat = spool.tile([128, 2], F32, tag=f"gstat{cot}",
                                   name=f"gstat_{cot}")
                # [128, 8, 2] -> sum over ranks per stat
                nc.vector.reduce_sum(
                    gstat[:], gath[:, cot].rearrange("p r s -> p s r"),
                    axis=mybir.AxisListType.X)
                mv = spool.tile([128, 2], F32, tag=f"mv{cot}",
                                name=f"mv_{cot}")
                mean, ey2e = mv[:, 0:1], mv[:, 1:2]
                var = spool.tile([128, 1], F32, tag=f"var{cot}",
                                 name=f"var_{cot}")
                r0 = spool.tile([128, 1], F32, tag=f"r0{cot}",
                                name=f"r0_{cot}")
                t1 = spool.tile([128, 1], F32, tag=f"t1{cot}",
                                name=f"t1_{cot}")
                sc = spool.tile([128, 1], F32, tag=f"sc{cot}",
                                name=f"sc_{cot}")
                bs = spool.tile([128, 1], F32, tag=f"bs{cot}",
                                name=f"bs_{cot}")
                nc.vector.tensor_scalar_mul(mean, gstat[:, 0:1],
                                            1.0 / NTOT_PIX)
                nc.vector.tensor_scalar(ey2e, gstat[:, 1:2],
                                        1.0 / NTOT_PIX, BN_EPS,
                                        op0=mybir.AluOpType.mult,
                                        op1=mybir.AluOpType.add)
                nc.vector.tensor_tensor(var[:], mean, mean,
                                        op=mybir.AluOpType.mult)
                nc.vector.tensor_tensor(var[:], ey2e, var[:],
                                        op=mybir.AluOpType.subtract)
                # inv = rsqrt(var+eps): sqrt(1/v) then one Newton step
                nc.vector.reciprocal(r0[:], var[:])
                nc.scalar.sqrt(r0[:], r0[:])
                nc.vector.tensor_tensor(t1[:], r0[:], r0[:],
                                        op=mybir.AluOpType.mult)
                nc.vector.tensor_tensor(t1[:], t1[:], var[:],
                                        op=mybir.AluOpType.mult)
                nc.vector.tensor_scalar(t1[:], t1[:], -0.5, 1.5,
                                        op0=mybir.AluOpType.mult,
                                        op1=mybir.AluOpType.add)
                nc.vector.tensor_tensor(r0[:], r0[:], t1[:],
                                        op=mybir.AluOpType.mult)
                nc.vector.tensor_tensor(sc[:], gb_g[:, cot:cot + 1], r0[:],
                                        op=mybir.AluOpType.mult)
                nc.vector.tensor_tensor(t1[:], mean, sc[:],
                                        op=mybir.AluOpType.mult)
                nc.vector.tensor_tensor(bs[:], gb_b[:, cot:cot + 1], t1[:],
                                        op=mybir.AluOpType.subtract)
                scbs[cot] = (sc, bs)

            def emit_norm(cot):
                """normalize + store in half-image tiles on DVE/ACT."""
                sc, bs = scbs[cot]
                for i, (n, hh) in enumerate(
                        (n, hh) for n in range(NL) for hh in range(2)):
                    ost = opool.tile([128, HHALF, W], F32, tag="ost", bufs=4,
                                     name=f"ost_{cot}_{n}_{hh}")
                    yv = ys[:, cot, n, HLEN * hh:HLEN * (hh + 1)].rearrange(
                        "p (r q) -> p r q", q=W)
                    if i % 2 == 1:
                        nc.scalar.activation(
                            ost[:], yv,
                            mybir.ActivationFunctionType.Identity,
                            bias=bs[:], scale=sc[:])
                    else:
                        nc.vector.tensor_scalar(
                            ost[:], yv, sc[:], bs[:],
                            op0=mybir.AluOpType.mult,
                            op1=mybir.AluOpType.add)
                    nc.sync.dma_start(
                        o[n, cot * 128:(cot + 1) * 128,
                          HHALF * hh:HHALF * (hh + 1), :],
                        ost[:])

            # ================= conv schedule =================
            # image 0 image-major; images 1-3 cot-major. cot0's stats +
            # norm + store overlap the cot1 conv stream.
            conv_group(0, 0, rev=True)
            emit_ssq(0, 0)
            conv_group(0, 1)
            # img3 reuses z slot 0: its sign (ACT) is emitted only after
            # image 0's conv — program order defines the WAR here.
            build_z(3)
            emit_ssq(0, 1)

            conv_group(1, 0)
            emit_ssq(1, 0)
            conv_group(2, 0)
            emit_ssq(2, 0)
            conv_group(3, 0)

            emit_stats(0)
            emit_finalize(0)
            emit_norm(0)

            conv_group(1, 1)
            emit_ssq(1, 1)
            conv_group(2, 1)
            emit_ssq(2, 1)
            conv_group(3, 1)

            emit_stats(1)
            emit_finalize(1)
            emit_norm(1)

    nc.compile()
    return nc


_CACHE: dict = {}


def _get_nc():
    key = "proxy" if os.environ.get("BK_TIMING_PROXY") == "1" else "real"
    if key not in _CACHE:
        _CACHE[key] = _build(timing_proxy=(key == "proxy"))
    return _CACHE[key]


def kernel(x, w, gamma, beta):
    x = np.ascontiguousarray(np.asarray(x, dtype=np.float32))
    w = np.asarray(w, dtype=np.float32)
    gamma = np.ascontiguousarray(np.asarray(gamma, dtype=np.float32))
    beta = np.ascontiguousarray(np.asarray(beta, dtype=np.float32))
    # host-side layout only (no math): [co,ci,kh,kw] -> [ci, kh*kw, co]
    w_t = np.ascontiguousarray(w.transpose(1, 2, 3, 0).reshape(CI, 9, CO))

    nc = _get_nc()
    in_maps = [
        {"xs": x[NL * c:NL * (c + 1)], "wt": w_t, "gamma": gamma, "beta": beta}
        for c in range(N_CORES)
    ]
    res = bass_utils.run_bass_kernel_spmd(
        nc, in_maps, core_ids=list(range(N_CORES)))
    return np.concatenate([res.results[c]["o"] for c in range(N_CORES)], axis=0)


# revision 11
# speedup vs baseline: 1.1934x; 1.0703x over previous
"""Binary conv (XNOR-style) 3x3 + sync-BN on 8 Trainium2 NeuronCores.

Problem: x[32,256,56,56], w[256,256,3,3] -> sign(x) conv sign(w), pad 1,
then BatchNorm (training mode, global batch stats) with gamma/beta.

Sharding: data-parallel over batch (4 images per core, 8 cores). BN batch
stats are made global with a tiny (2 KB) AllGather of per-channel
sum / sum-of-squares (sync-BN), so the result matches single-device math.

Per-core kernel (v3):
  - conv as shifted-window implicit GEMM on a zero-padded [58*58] fp8
    buffer; DoubleRow fp8 matmuls contract all 256 input channels at once.
    Moving operand is a 4D windowed AP [ci, 2, 8 rows stride 58, 56 cols]
    so each matmul computes 448 *valid* columns (no padded-column waste).
  - taps outer / chunks inner in 4+3 half-groups; consecutive groups use
    disjoint PSUM bank quads (bufs=8) so bank WAR never stalls the PE.
    One LDWEIGHTS per tap hides behind matmuls (background weight buffer).
  - startup: ~24 junk matmuls on a zeroed tile warm the PE HAM clock-gate;
    image 0 is DMA'd in row-quarters and binarized on DVE as
    (x>=0)-0.5 = +-0.5 (fp8-exact; rescaled 2x at PSUM evac) fully before
    its conv starts -- writing a z buffer the PE is streaming runs 10-20x
    slow, so build and consume never overlap a buffer. Weights are DMA'd
    + signed (ACT) in 3-tap pieces interleaved with the quarters; z pad
    borders memset once per buffer (interiors are overwritten per image).
  - images 1-3 run cot-major (img1..3 cot0, then img1..3 cot1) so the
    cot0 BN stats AllGather + finalize + normalize + 6.4MB output DMA all
    overlap the cot1 conv stream; only cot1's output write is a tail.
    All signs are emitted early on ACT (engine FIFOs are NOT reordered).
  - per-channel sum via DVE evac accum; sum(y^2)/64 via ACT Square with
    fp32 accumulator (big pass per img-cot; final image per-chunk so the
    stats tail is short); AllGather over 8 cores; rsqrt via
    reciprocal+sqrt+Newton; y*scale+bias on DVE/ACT in half-image tiles
    -> fp32 out.
"""

import os
import numpy as np

import concourse.bass as bass
import concourse.mybir as mybir
import concourse.tile as tile
from concourse import bacc
from concourse import bass_utils

F32 = mybir.dt.float32
F16 = mybir.dt.float16
F8 = mybir.dt.float8e4

N_CORES = 8
NL = 4            # images per core
CI = 256          # input channels
CO = 256          # output channels
H = W = 56
HP = 58           # padded row length
CR = 8            # image rows per chunk
NCHUNK = 7        # 7 chunks x 8 rows = 56 rows
VCHUNK = CR * W   # 448 matmul columns per chunk (all valid)
VLEN = NCHUNK * VCHUNK  # 3136
ZROWS = 58        # padded rows (top pad + 56 + bottom pad)
ZPAD = 3376       # fp8 per-ci-tile stride; >= 58*58+2 read slack, %16==0
HHALF = 28        # output norm/store granularity (half image rows)
HLEN = HHALF * W  # 1568
NTOT_PIX = 32 * H * W    # BN normalizer (full batch)
BN_EPS = 1e-5
SSQ_SCALE = 1.0 / 64.0  # keep y^2/64 in fp16 range in the junk output
DR = mybir.MatmulPerfMode.DoubleRow


def _build(timing_proxy: bool = False):
    nc = bacc.Bacc("TRN2", target_bir_lowering=False, debug=False,
                   num_devices=N_CORES)

    xs = nc.dram_tensor("xs", [NL, CI, H, W], F32, kind="ExternalInput").ap()
    wt = nc.dram_tensor("wt", [CI, 9, CO], F32, kind="ExternalInput").ap()
    gamma = nc.dram_tensor("gamma", [CO], F32, kind="ExternalInput").ap()
    beta = nc.dram_tensor("beta", [CO], F32, kind="ExternalInput").ap()
    o = nc.dram_tensor("o", [NL, CO, H, W], F32, kind="ExternalOutput").ap()

    with tile.TileContext(nc) as tc:
        with (
            tc.tile_pool(name="wpool", bufs=1) as wpool,
            tc.tile_pool(name="xpool", bufs=1) as xpool,
            tc.tile_pool(name="zpool", bufs=1) as zpool,
            tc.tile_pool(name="ypool", bufs=1) as ypool,
            tc.tile_pool(name="spool", bufs=1) as spool,
            tc.tile_pool(name="jpool", bufs=1) as jpool,
            tc.tile_pool(name="opool", bufs=1) as opool,
            tc.tile_pool(name="psum", bufs=8, space="PSUM") as psum_pool,
            tc.tile_pool(name="dram", bufs=1, space="DRAM") as dram,
        ):
            # ---- PE warmup: junk matmuls on a zeroed tile keep the HAM
            # clock-gate busy from ~7us so the real stream starts at 2.4GHz.
            # They draw PSUM banks from the same rotation as the conv accs.
            jk = spool.tile([128, 2, 256], F8, tag="jk")
            nc.vector.memset(jk[:], 0.0)
            for i in range(26):
                jkacc = psum_pool.tile([128, 256], F32, tag="acc",
                                       name=f"jkacc_{i}")
                nc.tensor.matmul(jkacc[:], jk[:, :, 0:128], jk[:], start=True,
                                 stop=True, perf_mode=DR)

            # preload the sqrt ACT table set off the critical path
            sqwarm = spool.tile([128, 1], F32, tag="sqwarm")
            nc.vector.memset(sqwarm[:], 1.0)
            nc.scalar.sqrt(sqwarm[:], sqwarm[:])

            gath = spool.tile([128, 2, N_CORES, 2], F32, tag="gath")
            nc.vector.memset(gath[:], 0.0)  # proxy mode only fills rank 0

            # ---- weights: DMA + ACT sign in 3-tap pieces, interleaved with
            # image-0 quarters so tap t is signed before the sweep needs it
            w_f32 = wpool.tile([128, 2, 9, CO], F32, tag="wf32")
            w_bin = wpool.tile([128, 2, 9, CO], F8, tag="wbin")
            wt_r = wt.rearrange("(ct p) t co -> p ct t co", p=128)

            # ---- z: 3 persistent padded fp8 buffers [p, buf, ci_tile, flat]
            # imgs 0,3 share slot 0; img1 slot 1; img2 slot 2. Pad borders
            # (row 0, row 57+slack, col triples) memset once; interior is
            # fully rewritten per image.
            zz = zpool.tile([128, 3, 2, ZPAD], F8, tag="zz")

            def pad_memset(b):
                nc.gpsimd.memset(zz[:, b, :, 0:HP], 0.0)
                trip = zz[:, b, :, 57:57 + 57 * HP].rearrange(
                    "p k (r t) -> p k r t", t=HP)[:, :, :, 0:3]
                nc.gpsimd.memset(trip, 0.0)
                nc.gpsimd.memset(zz[:, b, :, 57 * HP:ZPAD], 0.0)

            nc.sync.dma_start(w_f32[:, :, 0:3, :], wt_r[:, :, 0:3, :])

            # image 0: row-quarter DMAs, binarize on DVE as (x>=0)-0.5.
            # z0 is built completely before image 0's conv begins.
            pad_memset(0)
            QR = 14
            z58_0 = {ct: zz[:, 0, ct, 0:ZROWS * HP].rearrange(
                "p (r q) -> p r q", q=HP) for ct in range(2)}
            for q in range(4):
                for ct in range(2):
                    xq = xpool.tile([128, QR, W], F32, tag="xq", bufs=8,
                                    name=f"xq_{q}_{ct}")
                    nc.sync.dma_start(
                        xq[:], xs[0, ct * 128:(ct + 1) * 128,
                                  QR * q:QR * (q + 1)])
                    dst = z58_0[ct][:, 1 + QR * q:1 + QR * (q + 1), 2:58]
                    nc.vector.tensor_scalar(dst, xq[:], 0.0, 0.5,
                                            op0=mybir.AluOpType.is_ge,
                                            op1=mybir.AluOpType.subtract)
                if q == 0:
                    nc.sync.dma_start(w_f32[:, :, 3:6, :], wt_r[:, :, 3:6, :])
                if q == 1:
                    nc.sync.dma_start(w_f32[:, :, 6:9, :], wt_r[:, :, 6:9, :])
            pad_memset(1)
            pad_memset(2)

            nc.scalar.sign(w_bin[:, :, 0:1, :], w_f32[:, :, 0:1, :])
            nc.scalar.sign(w_bin[:, :, 1:3, :], w_f32[:, :, 1:3, :])
            nc.scalar.sign(w_bin[:, :, 3:6, :], w_f32[:, :, 3:6, :])
            nc.scalar.sign(w_bin[:, :, 6:9, :], w_f32[:, :, 6:9, :])

            # gamma/beta per-partition: channel c = t*128 + p
            gb_g = spool.tile([128, 2], F32, tag="gb_g")
            gb_b = spool.tile([128, 2], F32, tag="gb_b")

            # ---- persistent state ----
            ys = ypool.tile([128, 2, NL, VLEN], F16, tag="ys")
            sums = spool.tile([128, 2, NL, NCHUNK], F32, tag="sums")
            ssqa = spool.tile([128, 2, NL - 1 + NCHUNK], F32, tag="ssqa")

            jk2 = spool.tile([128, NL * NCHUNK], F32, tag="jk2")
            sums_b = spool.tile([128, 2, NL, NCHUNK], F32, tag="sums_b")
            scbs = {}

            ZSLOT = {0: 0, 1: 1, 2: 2, 3: 0}

            def build_z(n):
                """images 1-3: whole-image DMA then ACT sign into z."""
                for ct in range(2):
                    xst = xpool.tile([128, H, W], F32, tag="xst", bufs=3,
                                     name=f"xst_{n}_{ct}")
                    nc.sync.dma_start(xst[:], xs[n, ct * 128:(ct + 1) * 128])
                    z58 = zz[:, ZSLOT[n], ct, 0:ZROWS * HP].rearrange(
                        "p (r q) -> p r q", q=HP)
                    nc.scalar.sign(z58[:, 1:57, 2:58], xst[:])

            build_z(1)
            nc.sync.dma_start(gb_g[:], gamma.rearrange("(t p) -> p t", p=128))
            nc.sync.dma_start(gb_b[:], beta.rearrange("(t p) -> p t", p=128))
            build_z(2)

            def zwin(n, c, kh, kw):
                """Moving operand for chunk c, tap (kh,kw): [p, 2, 8, 56]
                rows stride 58 in the padded buffer."""
                base = HP * (CR * c + kh) + 1 + kw
                zs = zz[:, ZSLOT[n], :, base:base + 464]
                return zs.rearrange("p k (r q) -> p k r q", q=HP)[:, :, :, 0:W]

            def conv_group(n, cot, rev=False):
                """two 9-tap half-group sweeps (chunks 0-3, 4-6); evac to ys
                with per-chunk channel sums; img3 also squares per chunk.
                rev=True gates the first matmul on the last-written z
                quarter (never stream a z buffer while it's being built)."""
                cos = slice(cot * 128, (cot + 1) * 128)
                halves = ((6, 5, 4, 3), (2, 1, 0)) if rev else \
                    (range(0, 4), range(4, NCHUNK))
                for chunks in halves:
                    accs = {
                        c: psum_pool.tile([128, VCHUNK], F32, tag="acc",
                                          name=f"acc_{n}_{cot}_{c}")
                        for c in chunks
                    }
                    for t in range(9):
                        kh, kw = t // 3, t % 3
                        for c in chunks:
                            nc.tensor.matmul(
                                accs[c][:],
                                w_bin[:, :, t, cos],
                                zwin(n, c, kh, kw),
                                start=(t == 0), stop=(t == 8),
                                perf_mode=DR,
                            )
                    for c in chunks:
                        dst = ys[:, cot, n, VCHUNK * c:VCHUNK * (c + 1)]
                        nc.vector.tensor_scalar(
                            dst, accs[c][:], 2.0 if n == 0 else 1.0, 0.0,
                            op0=mybir.AluOpType.mult,
                            op1=mybir.AluOpType.add,
                            accum_out=sums[:, cot, n, c:c + 1])
                        if n == NL - 1:
                            # last image: chunk-granular squares so the
                            # post-conv stats tail is tiny (per cot)
                            junk = jpool.tile(
                                [128, VCHUNK], F16, tag="junkc", bufs=2,
                                name=f"junkc_{cot}_{c}")
                            nc.scalar.activation(
                                junk[:], dst,
                                mybir.ActivationFunctionType.Square,
                                scale=0.125,
                                accum_out=ssqa[:, cot, NL - 1 + c:NL + c])

            def emit_ssq(n, cot):
                # per-image sum(y^2): Square(y/8) w/ fp32 accum = sum(y^2)/64
                junk = jpool.tile([128, VLEN], F16, tag="junkb", bufs=2,
                                  name=f"junkb_{n}_{cot}")
                nc.scalar.activation(
                    junk[:], ys[:, cot, n, :],
                    mybir.ActivationFunctionType.Square,
                    scale=0.125,
                    accum_out=ssqa[:, cot, n:n + 1])

            def emit_stats(cot):
                # fold local stats: [sum, sum(y^2)/64] over imgs/chunks.
                # Cross-engine reads of accum_out tiles fault this HW, so
                # each accum tile gets an engine-local barrier copy first.
                cc_stage = spool.tile([128, 2], F32, tag=f"cc_stage{cot}",
                                      name=f"cc_stage_{cot}")
                nc.vector.tensor_copy(sums_b[:, cot], sums[:, cot])
                nc.scalar.activation(
                    jk2[:], sums_b[:, cot, :, :],
                    mybir.ActivationFunctionType.Copy,
                    accum_out=cc_stage[:, 0:1])
                nc.scalar.activation(
                    jk2[:, 0:NL - 1 + NCHUNK], ssqa[:, cot, :],
                    mybir.ActivationFunctionType.Copy,
                    scale=1.0 / SSQ_SCALE,
                    accum_out=cc_stage[:, 1:2])
                cc_stage2 = spool.tile([128, 2], F32, tag=f"cc_stage2{cot}",
                                       name=f"cc_stage2_{cot}")
                nc.scalar.copy(cc_stage2[:], cc_stage[:])
                cc_in = dram.tile([128, 2], F32, tag=f"cc_in{cot}",
                                  name=f"cc_in_{cot}")
                cc_out = dram.tile([N_CORES * 128, 2], F32,
                                   tag=f"cc_out{cot}", name=f"cc_out_{cot}")
                # stats DMAs go via the otherwise-idle GpSimd queue so they
                # never sit behind output-store descriptors on Sync
                nc.gpsimd.dma_start(cc_in[:], cc_stage2[:])
                if timing_proxy:
                    nc.gpsimd.dma_start(cc_out[0:128, :], cc_in[:])
                    nc.gpsimd.dma_start(gath[:, cot, 0, :], cc_out[0:128, :])
                else:
                    nc.gpsimd.collective_compute(
                        "AllGather",
                        mybir.AluOpType.bypass,
                        replica_groups=[list(range(N_CORES))],
                        ins=[cc_in.opt()],
                        outs=[cc_out.opt()],
                    )
                    nc.gpsimd.dma_start(
                        gath[:, cot],
                        cc_out.rearrange("(r p) s -> p r s", p=128))

            def emit_finalize(cot):
                gstat = spool.tile([128, 2], F32, tag=f"gstat{cot}",
                                   name=f"gstat_{cot}")
                # [128, 8, 2] -> sum over ranks per stat
                nc.vector.reduce_sum(
                    gstat[:], gath[:, cot].rearrange("p r s -> p s r"),
                    axis=mybir.AxisListType.X)
                mv = spool.tile([128, 2], F32, tag=f"mv{cot}",
                                name=f"mv_{cot}")
                mean, ey2e = mv[:, 0:1], mv[:, 1:2]
                var = spool.tile([128, 1], F32, tag=f"var{cot}",
                                 name=f"var_{cot}")
                r0 = spool.tile([128, 1], F32, tag=f"r0{cot}",
                                name=f"r0_{cot}")
                t1 = spool.tile([128, 1], F32, tag=f"t1{cot}",
                                name=f"t1_{cot}")
                sc = spool.tile([128, 1], F32, tag=f"sc{cot}",
                                name=f"sc_{cot}")
                bs = spool.tile([128, 1], F32, tag=f"bs{cot}",
                                name=f"bs_{cot}")
                nc.vector.tensor_scalar_mul(mean, gstat[:, 0:1],
                                            1.0 / NTOT_PIX)
                nc.vector.tensor_scalar(ey2e, gstat[:, 1:2],
                                        1.0 / NTOT_PIX, BN_EPS,
                                        op0=mybir.AluOpType.mult,
                                        op1=mybir.AluOpType.add)
                nc.vector.tensor_tensor(var[:], mean, mean,
                                        op=mybir.AluOpType.mult)
                nc.vector.tensor_tensor(var[:], ey2e, var[:],
                                        op=mybir.AluOpType.subtract)
                # inv = rsqrt(var+eps): sqrt(1/v) then one Newton step
                nc.vector.reciprocal(r0[:], var[:])
                nc.scalar.sqrt(r0[:], r0[:])
                nc.vector.tensor_tensor(t1[:], r0[:], r0[:],
                                        op=mybir.AluOpType.mult)
                nc.vector.tensor_tensor(t1[:], t1[:], var[:],
                                        op=mybir.AluOpType.mult)
                nc.vector.tensor_scalar(t1[:], t1[:], -0.5, 1.5,
                                        op0=mybir.AluOpType.mult,
                                        op1=mybir.AluOpType.add)
                nc.vector.tensor_tensor(r0[:], r0[:], t1[:],
                                        op=mybir.AluOpType.mult)
                nc.vector.tensor_tensor(sc[:], gb_g[:, cot:cot + 1], r0[:],
                                        op=mybir.AluOpType.mult)
                nc.vector.tensor_tensor(t1[:], mean, sc[:],
                                        op=mybir.AluOpType.mult)
                nc.vector.tensor_tensor(bs[:], gb_b[:, cot:cot + 1], t1[:],
                                        op=mybir.AluOpType.subtract)
                scbs[cot] = (sc, bs)

            def emit_norm(cot, lo, hi, dve_only=False):
                """normalize + store half-image tiles [lo, hi) of 8.
                cot0 runs DVE-only (the ACT queue must stay clear for the
                cot1 squares + stats fold on the critical path)."""
                sc, bs = scbs[cot]
                pairs = [(n, hh) for n in range(NL) for hh in range(2)]
                for i in range(lo, hi):
                    n, hh = pairs[i]
                    ost = opool.tile([128, HHALF, W], F32, tag="ost", bufs=4,
                                     name=f"ost_{cot}_{n}_{hh}")
                    yv = ys[:, cot, n, HLEN * hh:HLEN * (hh + 1)].rearrange(
                        "p (r q) -> p r q", q=W)
                    if i % 2 == 1 and not dve_only:
                        nc.scalar.activation(
                            ost[:], yv,
                            mybir.ActivationFunctionType.Identity,
                            bias=bs[:], scale=sc[:])
                    else:
                        nc.vector.tensor_scalar(
                            ost[:], yv, sc[:], bs[:],
                            op0=mybir.AluOpType.mult,
                            op1=mybir.AluOpType.add)
                    nc.sync.dma_start(
                        o[n, cot * 128:(cot + 1) * 128,
                          HHALF * hh:HHALF * (hh + 1), :],
                        ost[:])

            # ================= conv schedule =================
            # image 0 image-major; images 1-3 cot-major. cot0's stats +
            # norm + store overlap the cot1 conv stream.
            conv_group(0, 0, rev=True)
            emit_ssq(0, 0)
            conv_group(0, 1)
            # img3 reuses z slot 0: its sign (ACT) is emitted only after
            # image 0's conv — program order defines the WAR here.
            build_z(3)
            emit_ssq(0, 1)

            conv_group(1, 0)
            emit_ssq(1, 0)
            conv_group(2, 0)
            emit_ssq(2, 0)
            conv_group(3, 0)

            emit_stats(0)
            emit_finalize(0)

            # cot0 norm halves interleave the cot1 conv sections so the DVE
            # evac backlog never exceeds one PSUM generation
            conv_group(1, 1)
            emit_ssq(1, 1)
            emit_norm(0, 0, 4, dve_only=True)
            conv_group(2, 1)
            emit_ssq(2, 1)
            emit_norm(0, 4, 8, dve_only=True)
            conv_group(3, 1)

            emit_stats(1)
            emit_finalize(1)
            emit_norm(1, 0, 8)

    nc.compile()
    return nc


_CACHE: dict = {}


def _get_nc():
    key = "proxy" if os.environ.get("BK_TIMING_PROXY") == "1" else "real"
    if key not in _CACHE:
        _CACHE[key] = _build(timing_proxy=(key == "proxy"))
    return _CACHE[key]


def kernel(x, w, gamma, beta):
    x = np.ascontiguousarray(np.asarray(x, dtype=np.float32))
    w = np.asarray(w, dtype=np.float32)
    gamma = np.ascontiguousarray(np.asarray(gamma, dtype=np.float32))
    beta = np.ascontiguousarray(np.asarray(beta, dtype=np.float32))
    # host-side layout only (no math): [co,ci,kh,kw] -> [ci, kh*kw, co]
    w_t = np.ascontiguousarray(w.transpose(1, 2, 3, 0).reshape(CI, 9, CO))

    nc = _get_nc()
    in_maps = [
        {"xs": x[NL * c:NL * (c + 1)], "wt": w_t, "gamma": gamma, "beta": beta}
        for c in range(N_CORES)
    ]
    res = bass_utils.run_bass_kernel_spmd(
        nc, in_maps, core_ids=list(range(N_CORES)))
    return np.concatenate([res.results[c]["o"] for c in range(N_CORES)], axis=0)


# revision 14
# speedup vs baseline: 1.2913x; 1.0821x over previous
"""Binary conv (XNOR-style) 3x3 + sync-BN on 8 Trainium2 NeuronCores.

Problem: x[32,256,56,56], w[256,256,3,3] -> sign(x) conv sign(w), pad 1,
then BatchNorm (training mode, global batch stats) with gamma/beta.

Sharding: data-parallel over batch (4 images per core, 8 cores). BN batch
stats are made global with a tiny (2 KB) AllGather of per-channel
sum / sum-of-squares (sync-BN), so the result matches single-device math.

Per-core kernel (v3):
  - conv as shifted-window implicit GEMM on a zero-padded [58*58] fp8
    buffer; DoubleRow fp8 matmuls contract all 256 input channels at once.
    Moving operand is a 4D windowed AP [ci, 2, 8 rows stride 58, 56 cols]
    so each matmul computes 448 *valid* columns (no padded-column waste).
  - taps outer / chunks inner in 4+3 half-groups; consecutive groups use
    disjoint PSUM bank quads (bufs=8) so bank WAR never stalls the PE.
    One LDWEIGHTS per tap hides behind matmuls (background weight buffer).
  - startup: ~24 junk matmuls on a zeroed tile warm the PE HAM clock-gate;
    image 0 is DMA'd in row-quarters and binarized on DVE as
    (x>=0)-0.5 = +-0.5 (fp8-exact; rescaled 2x at PSUM evac) fully before
    its conv starts -- writing a z buffer the PE is streaming runs 10-20x
    slow, so build and consume never overlap a buffer. Weights are DMA'd
    + signed (ACT) in 3-tap pieces interleaved with the quarters; z pad
    borders memset once per buffer (interiors are overwritten per image).
  - images 1-3 run cot-major (img1..3 cot0, then img1..3 cot1) so the
    cot0 BN stats AllGather + finalize + normalize + 6.4MB output DMA all
    overlap the cot1 conv stream; only cot1's output write is a tail.
    All signs are emitted early on ACT (engine FIFOs are NOT reordered).
  - per-channel sum via DVE evac accum; sum(y^2)/64 via ACT Square with
    fp32 accumulator (big pass per img-cot; final image per-chunk so the
    stats tail is short); AllGather over 8 cores; rsqrt via
    reciprocal+sqrt+Newton; y*scale+bias on DVE/ACT in half-image tiles
    -> fp32 out.
"""

import os
import numpy as np

import concourse.bass as bass
import concourse.mybir as mybir
import concourse.tile as tile
from concourse import bacc
from concourse import bass_utils

F32 = mybir.dt.float32
F16 = mybir.dt.float16
F8 = mybir.dt.float8e4

N_CORES = 8
NL = 4            # images per core
CI = 256          # input channels
CO = 256          # output channels
H = W = 56
HP = 58           # padded row length
CR = 8            # image rows per chunk
NCHUNK = 7        # 7 chunks x 8 rows = 56 rows
VCHUNK = CR * W   # 448 matmul columns per chunk (all valid)
VLEN = NCHUNK * VCHUNK  # 3136
ZROWS = 58        # padded rows (top pad + 56 + bottom pad)
ZPAD = 3376       # fp8 per-ci-tile stride; >= 58*58+2 read slack, %16==0
HHALF = 28        # output norm/store granularity (half image rows)
HLEN = HHALF * W  # 1568
NTOT_PIX = 32 * H * W    # BN normalizer (full batch)
BN_EPS = 1e-5
SSQ_SCALE = 1.0 / 64.0  # keep y^2/64 in fp16 range in the junk output
DR = mybir.MatmulPerfMode.DoubleRow


def _build(timing_proxy: bool = False):
    nc = bacc.Bacc("TRN2", target_bir_lowering=False, debug=False,
                   num_devices=N_CORES)

    xs = nc.dram_tensor("xs", [NL, CI, H, W], F32, kind="ExternalInput").ap()
    wt = nc.dram_tensor("wt", [CI, 9, CO], F32, kind="ExternalInput").ap()
    gamma = nc.dram_tensor("gamma", [CO], F32, kind="ExternalInput").ap()
    beta = nc.dram_tensor("beta", [CO], F32, kind="ExternalInput").ap()
    o = nc.dram_tensor("o", [NL, CO, H, W], F32, kind="ExternalOutput").ap()

    with tile.TileContext(nc) as tc:
        with (
            tc.tile_pool(name="wpool", bufs=1) as wpool,
            tc.tile_pool(name="xpool", bufs=1) as xpool,
            tc.tile_pool(name="zpool", bufs=1) as zpool,
            tc.tile_pool(name="ypool", bufs=1) as ypool,
            tc.tile_pool(name="spool", bufs=1) as spool,
            tc.tile_pool(name="jpool", bufs=1) as jpool,
            tc.tile_pool(name="opool", bufs=1) as opool,
            tc.tile_pool(name="psum", bufs=8, space="PSUM") as psum_pool,
            tc.tile_pool(name="dram", bufs=1, space="DRAM") as dram,
        ):
            # ---- PE warmup: junk matmuls on a zeroed tile keep the HAM
            # clock-gate busy from ~7us so the real stream starts at 2.4GHz.
            # They draw PSUM banks from the same rotation as the conv accs.
            jk = spool.tile([128, 2, 448], F8, tag="jk")
            nc.vector.memset(jk[:], 0.0)
            for i in range(36):
                jkacc = psum_pool.tile([128, 448], F32, tag="acc",
                                       name=f"jkacc_{i}")
                nc.tensor.matmul(jkacc[:], jk[:, :, 0:128], jk[:], start=True,
                                 stop=True, perf_mode=DR)

            # preload the sqrt ACT table set off the critical path
            sqwarm = spool.tile([128, 1], F32, tag="sqwarm")
            nc.vector.memset(sqwarm[:], 1.0)
            nc.scalar.sqrt(sqwarm[:], sqwarm[:])

            gath = spool.tile([128, 2, N_CORES, 2], F32, tag="gath")
            nc.vector.memset(gath[:], 0.0)  # proxy mode only fills rank 0

            # ---- weights: DMA + ACT sign in 3-tap pieces, interleaved with
            # image-0 quarters so tap t is signed before the sweep needs it
            w_f32 = wpool.tile([128, 2, 9, CO], F32, tag="wf32")
            w_bin = wpool.tile([128, 2, 9, CO], F8, tag="wbin")
            wt_r = wt.rearrange("(ct p) t co -> p ct t co", p=128)

            # ---- z: 3 persistent padded fp8 buffers [p, buf, ci_tile, flat]
            # imgs 0,3 share slot 0; img1 slot 1; img2 slot 2. Pad borders
            # (row 0, row 57+slack, col triples) memset once; interior is
            # fully rewritten per image.
            zz = zpool.tile([128, 3, 2, ZPAD], F8, tag="zz")

            def pad_memset(b):
                nc.gpsimd.memset(zz[:, b, :, 0:HP], 0.0)
                trip = zz[:, b, :, 57:57 + 57 * HP].rearrange(
                    "p k (r t) -> p k r t", t=HP)[:, :, :, 0:3]
                nc.gpsimd.memset(trip, 0.0)
                nc.gpsimd.memset(zz[:, b, :, 57 * HP:ZPAD], 0.0)

            nc.sync.dma_start(w_f32[:, :, 0:3, :], wt_r[:, :, 0:3, :])

            # image 0: row-quarter DMAs, binarize on DVE as (x>=0)-0.5.
            # z0 is built completely before image 0's conv begins.
            pad_memset(0)
            QR = 14
            z58_0 = {ct: zz[:, 0, ct, 0:ZROWS * HP].rearrange(
                "p (r q) -> p r q", q=HP) for ct in range(2)}
            for q in range(4):
                for ct in range(2):
                    xq = xpool.tile([128, QR, W], F32, tag="xq", bufs=8,
                                    name=f"xq_{q}_{ct}")
                    nc.sync.dma_start(
                        xq[:], xs[0, ct * 128:(ct + 1) * 128,
                                  QR * q:QR * (q + 1)])
                    dst = z58_0[ct][:, 1 + QR * q:1 + QR * (q + 1), 2:58]
                    nc.vector.tensor_scalar(dst, xq[:], 0.0, 0.5,
                                            op0=mybir.AluOpType.is_ge,
                                            op1=mybir.AluOpType.subtract)
            # taps 3-8 transfer only after all of x0 — image 0's first half
            # group (chunks 6..3) gates on the last quarter, so x0 bytes are
            # the startup critical path, not the late taps
            nc.sync.dma_start(w_f32[:, :, 3:9, :], wt_r[:, :, 3:9, :])
            pad_memset(1)
            pad_memset(2)

            nc.scalar.sign(w_bin[:, :, 0:1, :], w_f32[:, :, 0:1, :])
            nc.scalar.sign(w_bin[:, :, 1:3, :], w_f32[:, :, 1:3, :])
            nc.scalar.sign(w_bin[:, :, 3:6, :], w_f32[:, :, 3:6, :])
            nc.scalar.sign(w_bin[:, :, 6:9, :], w_f32[:, :, 6:9, :])

            # gamma/beta per-partition: channel c = t*128 + p
            gb_g = spool.tile([128, 2], F32, tag="gb_g")
            gb_b = spool.tile([128, 2], F32, tag="gb_b")

            # ---- persistent state ----
            ys = ypool.tile([128, 2, NL, VLEN], F16, tag="ys")
            sums = spool.tile([128, 2, NL, NCHUNK], F32, tag="sums")
            ssqa = spool.tile([128, 2, NL - 1 + NCHUNK], F32, tag="ssqa")

            jk2 = spool.tile([128, NL * NCHUNK], F32, tag="jk2")
            sums_b = spool.tile([128, 2, NL, NCHUNK], F32, tag="sums_b")
            scbs = {}

            ZSLOT = {0: 0, 1: 1, 2: 2, 3: 0}

            def build_z(n):
                """images 1-3: whole-image DMA then ACT sign into z."""
                for ct in range(2):
                    xst = xpool.tile([128, H, W], F32, tag="xst", bufs=3,
                                     name=f"xst_{n}_{ct}")
                    nc.sync.dma_start(xst[:], xs[n, ct * 128:(ct + 1) * 128])
                    z58 = zz[:, ZSLOT[n], ct, 0:ZROWS * HP].rearrange(
                        "p (r q) -> p r q", q=HP)
                    nc.scalar.sign(z58[:, 1:57, 2:58], xst[:])

            build_z(1)
            nc.sync.dma_start(gb_g[:], gamma.rearrange("(t p) -> p t", p=128))
            nc.sync.dma_start(gb_b[:], beta.rearrange("(t p) -> p t", p=128))
            build_z(2)

            def zwin(n, c, kh, kw):
                """Moving operand for chunk c, tap (kh,kw): [p, 2, 8, 56]
                rows stride 58 in the padded buffer."""
                base = HP * (CR * c + kh) + 1 + kw
                zs = zz[:, ZSLOT[n], :, base:base + 464]
                return zs.rearrange("p k (r q) -> p k r q", q=HP)[:, :, :, 0:W]

            def conv_group(n, cot, rev=False):
                """two 9-tap half-group sweeps (chunks 0-3, 4-6); evac to ys
                with per-chunk channel sums; img3 also squares per chunk.
                rev=True gates the first matmul on the last-written z
                quarter (never stream a z buffer while it's being built)."""
                cos = slice(cot * 128, (cot + 1) * 128)
                halves = ((6, 5, 4, 3), (2, 1, 0)) if rev else \
                    (range(0, 4), range(4, NCHUNK))
                for chunks in halves:
                    accs = {
                        c: psum_pool.tile([128, VCHUNK], F32, tag="acc",
                                          name=f"acc_{n}_{cot}_{c}")
                        for c in chunks
                    }
                    for t in range(9):
                        kh, kw = t // 3, t % 3
                        for c in chunks:
                            nc.tensor.matmul(
                                accs[c][:],
                                w_bin[:, :, t, cos],
                                zwin(n, c, kh, kw),
                                start=(t == 0), stop=(t == 8),
                                perf_mode=DR,
                            )
                    for c in chunks:
                        dst = ys[:, cot, n, VCHUNK * c:VCHUNK * (c + 1)]
                        nc.vector.tensor_scalar(
                            dst, accs[c][:], 2.0 if n == 0 else 1.0, 0.0,
                            op0=mybir.AluOpType.mult,
                            op1=mybir.AluOpType.add,
                            accum_out=sums[:, cot, n, c:c + 1])
                        if n == NL - 1:
                            # last image: chunk-granular squares so the
                            # post-conv stats tail is tiny (per cot)
                            junk = jpool.tile(
                                [128, VCHUNK], F16, tag="junkc", bufs=2,
                                name=f"junkc_{cot}_{c}")
                            nc.scalar.activation(
                                junk[:], dst,
                                mybir.ActivationFunctionType.Square,
                                scale=0.125,
                                accum_out=ssqa[:, cot, NL - 1 + c:NL + c])

            def emit_ssq(n, cot):
                # per-image sum(y^2): Square(y/8) w/ fp32 accum = sum(y^2)/64
                junk = jpool.tile([128, VLEN], F16, tag="junkb", bufs=2,
                                  name=f"junkb_{n}_{cot}")
                nc.scalar.activation(
                    junk[:], ys[:, cot, n, :],
                    mybir.ActivationFunctionType.Square,
                    scale=0.125,
                    accum_out=ssqa[:, cot, n:n + 1])

            def emit_stats(cot):
                # fold local stats: [sum, sum(y^2)/64] over imgs/chunks.
                # Cross-engine reads of accum_out tiles fault this HW, so
                # each accum tile gets an engine-local barrier copy first.
                cc_stage = spool.tile([128, 2], F32, tag=f"cc_stage{cot}",
                                      name=f"cc_stage_{cot}")
                nc.vector.tensor_copy(sums_b[:, cot], sums[:, cot])
                nc.scalar.activation(
                    jk2[:], sums_b[:, cot, :, :],
                    mybir.ActivationFunctionType.Copy,
                    accum_out=cc_stage[:, 0:1])
                nc.scalar.activation(
                    jk2[:, 0:NL - 1 + NCHUNK], ssqa[:, cot, :],
                    mybir.ActivationFunctionType.Copy,
                    scale=1.0 / SSQ_SCALE,
                    accum_out=cc_stage[:, 1:2])
                cc_stage2 = spool.tile([128, 2], F32, tag=f"cc_stage2{cot}",
                                       name=f"cc_stage2_{cot}")
                nc.scalar.copy(cc_stage2[:], cc_stage[:])
                cc_in = dram.tile([128, 2], F32, tag=f"cc_in{cot}",
                                  name=f"cc_in_{cot}")
                cc_out = dram.tile([N_CORES * 128, 2], F32,
                                   tag=f"cc_out{cot}", name=f"cc_out_{cot}")
                # stats DMAs go via the otherwise-idle GpSimd queue so they
                # never sit behind output-store descriptors on Sync
                nc.gpsimd.dma_start(cc_in[:], cc_stage2[:])
                if timing_proxy:
                    # 2-hop stand-in (SBUF->DRAM->SBUF): about the latency
                    # of the real 2KB AllGather per the NRT measurements
                    nc.gpsimd.dma_start(gath[:, cot, 0, :], cc_in[:])
                else:
                    nc.gpsimd.collective_compute(
                        "AllGather",
                        mybir.AluOpType.bypass,
                        replica_groups=[list(range(N_CORES))],
                        ins=[cc_in.opt()],
                        outs=[cc_out.opt()],
                    )
                    nc.gpsimd.dma_start(
                        gath[:, cot],
                        cc_out.rearrange("(r p) s -> p r s", p=128))

            def emit_finalize(cot):
                gstat = spool.tile([128, 2], F32, tag=f"gstat{cot}",
                                   name=f"gstat_{cot}")
                # [128, 8, 2] -> sum over ranks per stat
                nc.vector.reduce_sum(
                    gstat[:], gath[:, cot].rearrange("p r s -> p s r"),
                    axis=mybir.AxisListType.X)
                mv = spool.tile([128, 2], F32, tag=f"mv{cot}",
                                name=f"mv_{cot}")
                mean, ey2e = mv[:, 0:1], mv[:, 1:2]
                var = spool.tile([128, 1], F32, tag=f"var{cot}",
                                 name=f"var_{cot}")
                r0 = spool.tile([128, 1], F32, tag=f"r0{cot}",
                                name=f"r0_{cot}")
                t1 = spool.tile([128, 1], F32, tag=f"t1{cot}",
                                name=f"t1_{cot}")
                sc = spool.tile([128, 1], F32, tag=f"sc{cot}",
                                name=f"sc_{cot}")
                bs = spool.tile([128, 1], F32, tag=f"bs{cot}",
                                name=f"bs_{cot}")
                nc.vector.tensor_scalar_mul(mean, gstat[:, 0:1],
                                            1.0 / NTOT_PIX)
                nc.vector.tensor_scalar(ey2e, gstat[:, 1:2],
                                        1.0 / NTOT_PIX, BN_EPS,
                                        op0=mybir.AluOpType.mult,
                                        op1=mybir.AluOpType.add)
                nc.vector.tensor_tensor(var[:], mean, mean,
                                        op=mybir.AluOpType.mult)
                nc.vector.tensor_tensor(var[:], ey2e, var[:],
                                        op=mybir.AluOpType.subtract)
                # inv = rsqrt(var+eps): sqrt(1/v) then one Newton step
                nc.vector.reciprocal(r0[:], var[:])
                nc.scalar.sqrt(r0[:], r0[:])
                nc.vector.tensor_tensor(t1[:], r0[:], r0[:],
                                        op=mybir.AluOpType.mult)
                nc.vector.tensor_tensor(t1[:], t1[:], var[:],
                                        op=mybir.AluOpType.mult)
                nc.vector.tensor_scalar(t1[:], t1[:], -0.5, 1.5,
                                        op0=mybir.AluOpType.mult,
                                        op1=mybir.AluOpType.add)
                nc.vector.tensor_tensor(r0[:], r0[:], t1[:],
                                        op=mybir.AluOpType.mult)
                nc.vector.tensor_tensor(sc[:], gb_g[:, cot:cot + 1], r0[:],
                                        op=mybir.AluOpType.mult)
                nc.vector.tensor_tensor(t1[:], mean, sc[:],
                                        op=mybir.AluOpType.mult)
                nc.vector.tensor_tensor(bs[:], gb_b[:, cot:cot + 1], t1[:],
                                        op=mybir.AluOpType.subtract)
                scbs[cot] = (sc, bs)

            def emit_norm(cot, lo, hi, dve_only=False):
                """normalize + store half-image tiles [lo, hi) of 8.
                cot0 runs DVE-only (the ACT queue must stay clear for the
                cot1 squares + stats fold on the critical path)."""
                sc, bs = scbs[cot]
                pairs = [(n, hh) for n in range(NL) for hh in range(2)]
                for i in range(lo, hi):
                    n, hh = pairs[i]
                    ost = opool.tile([128, HHALF, W], F32, tag="ost", bufs=4,
                                     name=f"ost_{cot}_{n}_{hh}")
                    yv = ys[:, cot, n, HLEN * hh:HLEN * (hh + 1)].rearrange(
                        "p (r q) -> p r q", q=W)
                    if i % 2 == 1 and not dve_only:
                        nc.scalar.activation(
                            ost[:], yv,
                            mybir.ActivationFunctionType.Identity,
                            bias=bs[:], scale=sc[:])
                    else:
                        nc.vector.tensor_scalar(
                            ost[:], yv, sc[:], bs[:],
                            op0=mybir.AluOpType.mult,
                            op1=mybir.AluOpType.add)
                    nc.sync.dma_start(
                        o[n, cot * 128:(cot + 1) * 128,
                          HHALF * hh:HHALF * (hh + 1), :],
                        ost[:])

            # ================= conv schedule =================
            # image 0 image-major; images 1-3 cot-major. cot0's stats +
            # norm + store overlap the cot1 conv stream.
            conv_group(0, 0, rev=True)
            emit_ssq(0, 0)
            conv_group(0, 1)
            # img3 reuses z slot 0: its sign (ACT) is emitted only after
            # image 0's conv — program order defines the WAR here.
            build_z(3)
            emit_ssq(0, 1)

            conv_group(1, 0)
            emit_ssq(1, 0)
            conv_group(2, 0)
            emit_ssq(2, 0)
            conv_group(3, 0)

            emit_stats(0)
            emit_finalize(0)

            # cot0 norm halves interleave the cot1 conv sections so the DVE
            # evac backlog never exceeds one PSUM generation
            conv_group(1, 1)
            emit_ssq(1, 1)
            emit_norm(0, 0, 4, dve_only=True)
            conv_group(2, 1)
            emit_ssq(2, 1)
            emit_norm(0, 4, 8, dve_only=True)
            conv_group(3, 1)

            emit_stats(1)
            emit_finalize(1)
            emit_norm(1, 0, 8)

    nc.compile()
    return nc


_CACHE: dict = {}


def _get_nc():
    key = "proxy" if os.environ.get("BK_TIMING_PROXY") == "1" else "real"
    if key not in _CACHE:
        _CACHE[key] = _build(timing_proxy=(key == "proxy"))
    return _CACHE[key]


def kernel(x, w, gamma, beta):
    x = np.ascontiguousarray(np.asarray(x, dtype=np.float32))
    w = np.asarray(w, dtype=np.float32)
    gamma = np.ascontiguousarray(np.asarray(gamma, dtype=np.float32))
    beta = np.ascontiguousarray(np.asarray(beta, dtype=np.float32))
    # host-side layout only (no math): [co,ci,kh,kw] -> [ci, kh*kw, co]
    w_t = np.ascontiguousarray(w.transpose(1, 2, 3, 0).reshape(CI, 9, CO))

    nc = _get_nc()
    in_maps = [
        {"xs": x[NL * c:NL * (c + 1)], "wt": w_t, "gamma": gamma, "beta": beta}
        for c in range(N_CORES)
    ]
    res = bass_utils.run_bass_kernel_spmd(
        nc, in_maps, core_ids=list(range(N_CORES)))
    return np.concatenate([res.results[c]["o"] for c in range(N_CORES)], axis=0)
